# revision 5
# baseline (speedup 1.0000x reference)
"""Trainium2 Bass kernel for nn_BasicModel_47871705481510.

Math: per qubit i, with v_i = w_i + (x[0] if i==0 else x[1] if i==1 else 0):
  state_i = [cos(v_i/2), -i*sin(v_i/2)]^T   (Rx(w) @ Rx(theta1) |0> collapses
                                             to a single rotation by v_i)
  O_i     = cos(v_i)                         (real)

Device computes three f32 planes per qubit via the ACT Sin table (accurate
only for |arg| <= pi, hence the quarter/half-angle forms):
  sneg = sin(-v/2)            (= imag part of state[1])
  c    = 1 - 2*sin^2(v/4)     (= cos(v/2), real part of state[0])
  cosv = 1 - 2*sin^2(v/2)     (= cos(v) = O)
Host assembles the complex64 outputs (structural zeros / interleave only).

Sharding: data-parallel over qubits. 2,000,000 qubits padded to 8 * 128 * 2048
= 2,097,152; each of the 8 cores handles one contiguous [128, 2048] f32 shard.
x is passed as a [1,2] tensor: the real x to core 0 (whose shard holds qubits
0,1 at partition 0, cols 0,1), zeros to the other cores, so the SPMD program
is uniform.
"""

import numpy as np

N = 2_000_000
N_CORES = 8
P = 128
F = 2048
PER_CORE = P * F  # 262144
N_PAD = PER_CORE * N_CORES

CHUNK_F = 512
BUFS = 4

_cache = {}


def _build_nc():
    import concourse.bacc as bacc
    import concourse.mybir as mybir
    import concourse.tile as tile

    nc = bacc.Bacc(
        "TRN2", target_bir_lowering=False, debug=False, num_devices=N_CORES
    )
    w_in = nc.declare_dram_parameter("w", [P, F], mybir.dt.float32, isOutput=False)
    x_in = nc.declare_dram_parameter("x", [1, 2], mybir.dt.float32, isOutput=False)
    c_out = nc.declare_dram_parameter("c", [P, F], mybir.dt.float32, isOutput=True)
    s_out = nc.declare_dram_parameter("s", [P, F], mybir.dt.float32, isOutput=True)
    o_out = nc.declare_dram_parameter("o", [P, F], mybir.dt.float32, isOutput=True)

    SIN = mybir.ActivationFunctionType.Sin
    SQUARE = mybir.ActivationFunctionType.Square
    MULT = mybir.AluOpType.mult
    ADD = mybir.AluOpType.add
    f32 = mybir.dt.float32

    with tile.TileContext(nc) as tc:
        with tc.tile_pool(name="pool", bufs=BUFS) as pool:
            xt = pool.tile([1, 2], f32, tag="xt")
            nc.gpsimd.dma_start(xt[:], x_in[:])
            for j in range(F // CHUNK_F):
                sl = slice(j * CHUNK_F, (j + 1) * CHUNK_F)
                wt = pool.tile([P, CHUNK_F], f32, tag="wt")
                # DMA triggers spread over three issue paths: inputs on the
                # Scalar HWDGE queue, c/o outputs on the Sync HWDGE queue,
                # s outputs (+x) on the GpSimd SWDGE queue. A DIRECT2D issue
                # costs ~0.65us of sequencer time, so one queue can't
                # sustain the ~358 GB/s HBM floor alone.
                nc.scalar.dma_start(wt[:], w_in[:, sl])
                if j == 0:
                    # qubits 0,1 live at partition 0, cols 0,1 of core 0's shard
                    nc.vector.tensor_add(wt[0:1, 0:2], wt[0:1, 0:2], xt[0:1, 0:2])
                ut = pool.tile([P, CHUNK_F], f32, tag="ut")
                nc.scalar.activation(ut[:], wt[:], SIN, scale=0.25)  # sin(v/4)
                st = pool.tile([P, CHUNK_F], f32, tag="st")
                nc.scalar.activation(st[:], wt[:], SIN, scale=-0.5)  # -sin(v/2)
                u2 = pool.tile([P, CHUNK_F], f32, tag="u2")
                nc.vector.tensor_mul(u2[:], ut[:], ut[:])
                ct = pool.tile([P, CHUNK_F], f32, tag="ct")
                nc.vector.tensor_scalar(ct[:], u2[:], -2.0, 1.0, MULT, ADD)
                s2 = pool.tile([P, CHUNK_F], f32, tag="s2")
                nc.vector.tensor_mul(s2[:], st[:], st[:])
                ot = pool.tile([P, CHUNK_F], f32, tag="ot")
                nc.vector.tensor_scalar(ot[:], s2[:], -2.0, 1.0, MULT, ADD)
                nc.sync.dma_start(c_out[:, sl], ct[:])
                nc.gpsimd.dma_start(s_out[:, sl], st[:])
                nc.sync.dma_start(o_out[:, sl], ot[:])
    nc.finalize()
    return nc


def _get_nc():
    if "nc" not in _cache:
        _cache["nc"] = _build_nc()
    return _cache["nc"]


def _run(x, w, **spmd_kwargs):
    """Shard, run on 8 cores, return (c, sneg, cosv) full f32 vectors plus
    the raw BassKernelResults (for profiling from test harnesses)."""
    from concourse.bass_utils import run_bass_kernel_spmd

    x = np.ascontiguousarray(np.asarray(x, dtype=np.float32)).reshape(1, 2)
    w = np.asarray(w, dtype=np.float32).reshape(-1)
    assert w.shape[0] == N
    w_pad = np.zeros(N_PAD, dtype=np.float32)
    w_pad[:N] = w
    shards = w_pad.reshape(N_CORES, P, F)
    zero_x = np.zeros((1, 2), dtype=np.float32)
    in_maps = [
        {"w": shards[i], "x": (x if i == 0 else zero_x)} for i in range(N_CORES)
    ]
    res = run_bass_kernel_spmd(_get_nc(), in_maps, list(range(N_CORES)), **spmd_kwargs)
    c = np.concatenate([r["c"].reshape(-1) for r in res.results])[:N]
    sneg = np.concatenate([r["s"].reshape(-1) for r in res.results])[:N]
    cosv = np.concatenate([r["o"].reshape(-1) for r in res.results])[:N]
    return c, sneg, cosv, res


def kernel(x, w):
    c, sneg, cosv, _ = _run(x, w)
    state = np.zeros((N, 4), dtype=np.float32)
    state[:, 0] = c
    state[:, 3] = sneg
    state = state.view(np.complex64).reshape(N, 2, 1)
    O = np.zeros((N, 2), dtype=np.float32)
    O[:, 0] = cosv
    O = O.view(np.complex64).reshape(N, 1, 1)
    return state, O


# revision 8
# speedup vs baseline: 1.0476x; 1.0476x over previous
"""Trainium2 Bass kernel for nn_BasicModel_47871705481510.

Math: per qubit i, with v_i = w_i + (x[0] if i==0 else x[1] if i==1 else 0):
  state_i = [cos(v_i/2), -i*sin(v_i/2)]^T   (Rx(w) @ Rx(theta1) |0> collapses
                                             to a single rotation by v_i)
  O_i     = cos(v_i)                         (real)

Device computes three f32 planes per qubit via the ACT Sin table (accurate
only for |arg| <= pi, hence the quarter/half-angle forms):
  sneg = sin(-v/2)            (= imag part of state[1])
  c    = 1 - 2*sin^2(v/4)     (= cos(v/2), real part of state[0])
  cosv = 1 - 2*sin^2(v/2)     (= cos(v) = O)
Host assembles the complex64 outputs (structural zeros / interleave only).

Sharding: data-parallel over qubits. 2,000,000 qubits padded to 8 * 128 * 2048
= 2,097,152; each of the 8 cores handles one contiguous [128, 2048] f32 shard.
x is passed as a [1,2] tensor: the real x to core 0 (whose shard holds qubits
0,1 at partition 0, cols 0,1), zeros to the other cores, so the SPMD program
is uniform.
"""

import numpy as np

N = 2_000_000
N_CORES = 8
P = 128
F = 2048
PER_CORE = P * F  # 262144
N_PAD = PER_CORE * N_CORES

CHUNK_F = 1024
BUFS = 3

_cache = {}


def _build_nc():
    import concourse.bacc as bacc
    import concourse.mybir as mybir
    import concourse.tile as tile

    nc = bacc.Bacc(
        "TRN2", target_bir_lowering=False, debug=False, num_devices=N_CORES
    )
    w_in = nc.declare_dram_parameter("w", [P, F], mybir.dt.float32, isOutput=False)
    x_in = nc.declare_dram_parameter("x", [1, 2], mybir.dt.float32, isOutput=False)
    c_out = nc.declare_dram_parameter("c", [P, F], mybir.dt.float32, isOutput=True)
    s_out = nc.declare_dram_parameter("s", [P, F], mybir.dt.float32, isOutput=True)
    o_out = nc.declare_dram_parameter("o", [P, F], mybir.dt.float32, isOutput=True)

    SIN = mybir.ActivationFunctionType.Sin
    SQUARE = mybir.ActivationFunctionType.Square
    MULT = mybir.AluOpType.mult
    ADD = mybir.AluOpType.add
    f32 = mybir.dt.float32

    with tile.TileContext(nc) as tc:
        with tc.tile_pool(name="pool", bufs=BUFS) as pool:
            # Tiny dummy Sin first: forces the ~1.3us ACT_TABLE_LOAD for the
            # sin set to run at body start, overlapped with the first input
            # DMA, instead of serializing before the first real Sin.
            warm = pool.tile([1, 1], f32, tag="warm")
            zconst = nc.const_aps.tensor(0.0, (1, 1), f32)
            nc.scalar.activation(warm[:], zconst, SIN)
            xt = pool.tile([1, 2], f32, tag="xt")
            nc.gpsimd.dma_start(xt[:], x_in[:])
            for j in range(F // CHUNK_F):
                sl = slice(j * CHUNK_F, (j + 1) * CHUNK_F)
                wt = pool.tile([P, CHUNK_F], f32, tag="wt")
                # DMA triggers spread over three issue paths: inputs on the
                # Scalar HWDGE queue, c/o outputs on the Sync HWDGE queue,
                # s outputs (+x) on the GpSimd SWDGE queue. A DIRECT2D issue
                # costs ~0.65us of sequencer time, so one queue can't
                # sustain the ~358 GB/s HBM floor alone.
                nc.scalar.dma_start(wt[:], w_in[:, sl])
                if j == 0:
                    # qubits 0,1 live at partition 0, cols 0,1 of core 0's shard
                    nc.vector.tensor_add(wt[0:1, 0:2], wt[0:1, 0:2], xt[0:1, 0:2])
                ut = pool.tile([P, CHUNK_F], f32, tag="ut")
                nc.scalar.activation(ut[:], wt[:], SIN, scale=0.25)  # sin(v/4)
                st = pool.tile([P, CHUNK_F], f32, tag="st")
                nc.scalar.activation(st[:], wt[:], SIN, scale=-0.5)  # -sin(v/2)
                u2 = pool.tile([P, CHUNK_F], f32, tag="u2")
                nc.vector.tensor_mul(u2[:], ut[:], ut[:])
                ct = pool.tile([P, CHUNK_F], f32, tag="ct")
                nc.vector.tensor_scalar(ct[:], u2[:], -2.0, 1.0, MULT, ADD)
                s2 = pool.tile([P, CHUNK_F], f32, tag="s2")
                nc.scalar.activation(s2[:], st[:], SQUARE)
                ot = pool.tile([P, CHUNK_F], f32, tag="ot")
                nc.vector.tensor_scalar(ot[:], s2[:], -2.0, 1.0, MULT, ADD)
                nc.sync.dma_start(c_out[:, sl], ct[:])
                nc.gpsimd.dma_start(s_out[:, sl], st[:])
                nc.sync.dma_start(o_out[:, sl], ot[:])
    nc.finalize()
    return nc


def _get_nc():
    if "nc" not in _cache:
        _cache["nc"] = _build_nc()
    return _cache["nc"]


def _run(x, w, **spmd_kwargs):
    """Shard, run on 8 cores, return (c, sneg, cosv) full f32 vectors plus
    the raw BassKernelResults (for profiling from test harnesses)."""
    from concourse.bass_utils import run_bass_kernel_spmd

    x = np.ascontiguousarray(np.asarray(x, dtype=np.float32)).reshape(1, 2)
    w = np.asarray(w, dtype=np.float32).reshape(-1)
    assert w.shape[0] == N
    w_pad = np.zeros(N_PAD, dtype=np.float32)
    w_pad[:N] = w
    shards = w_pad.reshape(N_CORES, P, F)
    zero_x = np.zeros((1, 2), dtype=np.float32)
    in_maps = [
        {"w": shards[i], "x": (x if i == 0 else zero_x)} for i in range(N_CORES)
    ]
    res = run_bass_kernel_spmd(_get_nc(), in_maps, list(range(N_CORES)), **spmd_kwargs)
    c = np.concatenate([r["c"].reshape(-1) for r in res.results])[:N]
    sneg = np.concatenate([r["s"].reshape(-1) for r in res.results])[:N]
    cosv = np.concatenate([r["o"].reshape(-1) for r in res.results])[:N]
    return c, sneg, cosv, res


def kernel(x, w):
    c, sneg, cosv, _ = _run(x, w)
    state = np.zeros((N, 4), dtype=np.float32)
    state[:, 0] = c
    state[:, 3] = sneg
    state = state.view(np.complex64).reshape(N, 2, 1)
    O = np.zeros((N, 2), dtype=np.float32)
    O[:, 0] = cosv
    O = O.view(np.complex64).reshape(N, 1, 1)
    return state, O
